# revision 30
# baseline (speedup 1.0000x reference)
"""Sparse-attention Bass kernel for Trainium2 (8 NeuronCores, data-parallel over batch).

Per batch b:
    kx    = k @ Wk.T + bk                      [KL, H]
    qx    = q @ Wq.T + bq                      [QL, H]
    qw    = qx @ weight                        [QL, H]
    score = tanh(qw @ kx.T)                    [QL, KL]
    p     = exp(score + maskbias)              (maskbias = 0 / -50 prefix mask)
    attn  = p / colsum_k(p)                    [QL, KL]   (output 1)
    out   = attn @ (kx @ Wp.T) + bp            [QL, E]    (output 2, reassociated)

Everything on-chip is laid out so matmul contraction dims sit on SBUF
partitions; score/attn live transposed as [KL, QL] and the host transposes the
attn/out results back. Matmuls use float32r (fast fp32 path). B=16 batches,
2 per core.
"""
import sys

sys.path.insert(0, "/opt/trn_rl_repo")

import numpy as np

import concourse.bass as bass
import concourse.mybir as mybir
import concourse.tile as tile
from concourse import bacc
from concourse.bass_utils import run_bass_kernel_spmd

F32 = mybir.dt.float32
F32R = mybir.dt.float32r
AF = mybir.ActivationFunctionType

B, KL, QL, D = 16, 2048, 512, 1024
NCORES = 8
BPC = B // NCORES          # batches per core
nD = D // 128              # 8
nK = KL // 128             # 16
nQ = QL // 128             # 4
nC = KL // 512             # 4
MASK_NEG = -50.0

_compiled_nc = None
last_results = None


def _build_nc():
    nc = bacc.Bacc("TRN2", target_bir_lowering=False)

    # ---- DRAM I/O (per core) ----
    kT_d = nc.dram_tensor("kT", [BPC, nD, 128, KL], F32R, kind="ExternalInput")
    qT_d = nc.dram_tensor("qT", [BPC, nD, 128, QL], F32R, kind="ExternalInput")
    wkT_d = nc.dram_tensor("wkT", [nD, 128, D], F32R, kind="ExternalInput")
    wqT_d = nc.dram_tensor("wqT", [nD, 128, D], F32R, kind="ExternalInput")
    wgt_d = nc.dram_tensor("wgt", [nD, 128, D], F32R, kind="ExternalInput")
    wpT_d = nc.dram_tensor("wpT", [nD, 128, D], F32R, kind="ExternalInput")
    bk_d = nc.dram_tensor("bk", [128, nD], F32, kind="ExternalInput")
    bq_d = nc.dram_tensor("bq", [128, nD], F32, kind="ExternalInput")
    bp_d = nc.dram_tensor("bp", [128, nD], F32, kind="ExternalInput")
    maskb_d = nc.dram_tensor("maskb", [BPC, 128, nK], F32, kind="ExternalInput")
    ones_d = nc.dram_tensor("ones", [128, 128], F32R, kind="ExternalInput")
    identr_d = nc.dram_tensor("identr", [128, 128], F32R, kind="ExternalInput")
    # outputs, transposed: attnT[b, kt, p, q] = attn[b, q, kt*128+p]
    attn_o = nc.dram_tensor("attn_o", [BPC, nK, 128, QL], F32, kind="ExternalOutput")
    out_o = nc.dram_tensor("out_o", [BPC, nD, 128, QL], F32, kind="ExternalOutput")

    with tile.TileContext(nc) as tc:
        cpool = tc.alloc_tile_pool(name="consts", bufs=1)
        ones_t = cpool.tile([128, 128], F32R)
        gate_t = cpool.tile([1, 1], F32R)
        ident_r = cpool.tile([128, 128], F32R)
        nc.gpsimd.dma_start(ident_r[:], identr_d[:])
        bk_t = cpool.tile([128, nD], F32)
        bq_t = cpool.tile([128, nD], F32)
        bp_t = cpool.tile([128, nD], F32)
        nc.gpsimd.dma_start(ones_t[:], ones_d[:])
        nc.gpsimd.dma_start(bk_t[:], bk_d[:])
        nc.gpsimd.dma_start(bq_t[:], bq_d[:])
        nc.gpsimd.dma_start(bp_t[:], bp_d[:])

        for b in range(BPC):
            # persist across the whole batch iteration
            persist = tc.alloc_tile_pool(name=f"persist{b}", bufs=1)
            kxT = persist.tile([128, nD, KL], F32R)   # kx^T[h, k]
            qwT = persist.tile([128, nD, QL], F32R)   # qw^T[g, q]

            # Q-side (MM2/MM3) and K-side (MM1) are interleaved on the PE so
            # each one's DMA stalls are covered by the other's matmuls.
            # kT/wk stream on the gpsimd DMA queue, q-side on sync.
            kpool = tc.alloc_tile_pool(name=f"k{b}", bufs=1)
            qpool = tc.alloc_tile_pool(name=f"q{b}", bufs=1)
            psQ = tc.alloc_tile_pool(name=f"psq{b}", bufs=3, space="PSUM")
            psB = tc.alloc_tile_pool(name=f"psb{b}", bufs=1, space="PSUM")

            qT_td = []
            for dd in range(nD):
                qtd = qpool.tile([128, QL], F32R, tag="qtd", bufs=nD)
                nc.gpsimd.dma_start(qtd[:], qT_d[b, dd])
                qT_td.append(qtd)
            qxT = qpool.tile([128, nD, QL], F32R)

            def mm2_group(gi):
                h0 = gi * 4
                wq_a = qpool.tile([128, nD, 256], F32R, tag="wcol", bufs=4)
                nc.sync.dma_start(
                    wq_a[:], wqT_d[:, :, h0 * 128:h0 * 128 + 256].rearrange("n p k -> p n k"))
                wq_b = qpool.tile([128, nD, 256], F32R, tag="wcol", bufs=4)
                nc.sync.dma_start(
                    wq_b[:], wqT_d[:, :, h0 * 128 + 256:h0 * 128 + 512].rearrange("n p k -> p n k"))
                wq_ch = [wq_a, wq_b]
                pq = []
                for j in range(4):
                    pq_t = psQ.tile([128, QL], F32, tag=f"pq{j}", bufs=1)
                    pq.append(pq_t)
                for d in range(nD):
                    for j in range(4):
                        nc.tensor.matmul(pq[j][:],
                                         wq_ch[j // 2][:, d, (j % 2) * 128:(j % 2) * 128 + 128],
                                         qT_td[d][:],
                                         start=(d == 0), stop=(d == nD - 1))
                for j in range(4):
                    h = h0 + j
                    nc.scalar.add(qxT[:, h, :], pq[j][:], bq_t[:, h:h + 1])

            def mm3_chunk(gs):
                g0 = gs[0]
                for g in gs:
                    j = g - g0
                    if j % 2 == 0:
                        wg_ch = qpool.tile([128, nD, 256], F32R, tag="wcol", bufs=4)
                        nc.sync.dma_start(
                            wg_ch[:], wgt_d[:, :, g * 128:g * 128 + 256].rearrange("n p k -> p n k"))
                    pg = psQ.tile([128, QL], F32, tag=f"pq{j % 2}", bufs=1)
                    for h in range(nD):
                        nc.tensor.matmul(pg[:], wg_ch[:, h, (j % 2) * 128:(j % 2) * 128 + 128],
                                         qxT[:, h, :],
                                         start=(h == 0), stop=(h == nD - 1))
                    nc.vector.tensor_copy(qwT[:, g, :], pg[:])

            wk_hg = []

            def load_wk():
                for hg in range(2):
                    wk_g = kpool.tile([128, nD, 512], F32R, tag=f"wkhg{hg}", bufs=1)
                    nc.gpsimd.dma_start(
                        wk_g[:], wkT_d[:, :, hg * 512:(hg + 1) * 512].rearrange("n p k -> p n k"))
                    wk_hg.append(wk_g)

            def mm1_chunk(c):
                cs = slice(c * 512, (c + 1) * 512)
                kT_c = []
                for dh in range(2):
                    kT_h = kpool.tile([128, nD // 2, 512], F32R, tag="ktc", bufs=3)
                    nc.gpsimd.dma_start(
                        kT_h[:], kT_d[b, dh * 4:(dh + 1) * 4, :, cs].rearrange("n p k -> p n k"))
                    kT_c.append(kT_h)
                for hg in range(2):
                    for j in range(4):
                        h = hg * 4 + j
                        pk = psB.tile([128, 512], F32, tag=f"pk{j}", bufs=1)
                        for dh in range(2):
                            for dd in range(4):
                                nc.tensor.matmul(
                                    pk[:],
                                    wk_hg[hg][:, dh * 4 + dd, j * 128:(j + 1) * 128],
                                    kT_c[dh][:, dd, :],
                                    start=(dh == 0 and dd == 0),
                                    stop=(dh == 1 and dd == 3))
                        nc.scalar.add(kxT[:, h, cs], pk[:], bk_t[:, h:h + 1])

            mm2_group(0)
            load_wk()
            mm2_group(1)
            mm3_chunk(list(range(0, 4)))
            mm1_chunk(0)
            mm3_chunk(list(range(4, 8)))
            mm1_chunk(1)
            mm1_chunk(2)
            mm1_chunk(3)

            psB.release()
            psQ.release()
            qpool.release()
            kpool.release()

            # ---------------- MM4: score^T[k, q] + masked exp + Z
            oupool = tc.alloc_tile_pool(name=f"ou{b}", bufs=1)
            out_uT = oupool.tile([128, nD, QL], F32R)
            rzb = oupool.tile([128, QL], F32)          # 1/Z broadcast to 128 parts
            spool = tc.alloc_tile_pool(name=f"s{b}", bufs=1)
            p_t = spool.tile([128, nK, QL], F32R)      # exp(tanh(score)+maskb)
            maskb_t = spool.tile([128, nK], F32)
            nc.sync.dma_start(maskb_t[:], maskb_d[b])

            psZ = tc.alloc_tile_pool(name=f"psz{b}", bufs=1, space="PSUM")
            zacc = psZ.tile([128, QL], F32, tag="z", bufs=1)
            psS = tc.alloc_tile_pool(name=f"pss{b}", bufs=6, space="PSUM")
            for kt in range(nK):
                sc = psS.tile([128, QL], F32, tag="sc")
                ks = slice(kt * 128, (kt + 1) * 128)
                for g in range(nD):
                    nc.tensor.matmul(sc[:], kxT[:, g, ks], qwT[:, g, :],
                                     start=(g == 0), stop=(g == nD - 1))
                th = spool.tile([128, QL], F32, tag="th", bufs=2)
                nc.scalar.activation(th[:], sc[:], AF.Tanh)
                nc.scalar.activation(p_t[:, kt, :], th[:], AF.Exp,
                                     bias=maskb_t[:, kt:kt + 1])
                nc.tensor.matmul(zacc[:], ones_t[:], p_t[:, kt, :],
                                 start=(kt == 0), stop=(kt == nK - 1))
            psS.release()
            nc.vector.reciprocal(rzb[:], zacc[:])

            # attn^T output = p * (1/Z)
            for kt in range(nK):
                stage = spool.tile([128, QL], F32, tag="stage", bufs=3)
                nc.vector.tensor_mul(stage[:], p_t[:, kt, :].bitcast(F32), rzb[:])
                nc.sync.dma_start(attn_o[b, kt], stage[:])

            wp_chunks = []
            for ch in range(2):
                wp_c = oupool.tile([128, nD, 512], F32R, tag="wpc", bufs=2)
                nc.sync.dma_start(
                    wp_c[:], wpT_d[:, :, ch * 512:(ch + 1) * 512].rearrange("n p k -> p n k"))
                wp_chunks.append(wp_c)

            # ---------------- out_u^T[h,q] = sum_k kx[k,h] p[k,q]  (kx via PE
            # transpose of kx^T, two h-halves), then out2^T = Wp^T-contract + bp
            kxpool = tc.alloc_tile_pool(name=f"kx{b}", bufs=1)
            psT5 = tc.alloc_tile_pool(name=f"pst{b}", bufs=1, space="PSUM")
            for hh in range(2):
                kx = kxpool.tile([128, nK, 512], F32R, tag="kx", bufs=1)
                for kt in range(nK):
                    ks = slice(kt * 128, (kt + 1) * 128)
                    tp = psT5.tile([128, 512], F32R, tag="tp", bufs=3)
                    for j in range(4):
                        nc.tensor.transpose(tp[:, j * 128:(j + 1) * 128],
                                            kxT[:, hh * 4 + j, ks], ident_r[:])
                    if kt % 2 == 0:
                        nc.vector.tensor_copy(kx[:, kt, :], tp[:])
                    else:
                        nc.scalar.copy(kx[:, kt, :], tp[:])
                po = []
                for j in range(4):
                    po_t = psT5.tile([128, QL], F32, tag=f"po{j}", bufs=1)
                    po.append(po_t)
                for kt in range(nK):
                    for j in range(4):
                        nc.tensor.matmul(po[j][:], kx[:, kt, j * 128:(j + 1) * 128],
                                         p_t[:, kt, :],
                                         start=(kt == 0), stop=(kt == nK - 1))
                for j in range(4):
                    h = hh * 4 + j
                    if j % 2 == 0:
                        nc.vector.tensor_copy(out_uT[:, h, :], po[j][:])
                    else:
                        nc.scalar.copy(out_uT[:, h, :], po[j][:])
            psT5.release()
            kxpool.release()
            psZ.release()
            spool.release()

            # out2^T[e, q] = sum_h WpT[h, e] out_u^T[h, q], * rz + bp
            wpool = tc.alloc_tile_pool(name=f"wp{b}", bufs=1)
            ps6 = tc.alloc_tile_pool(name=f"ps6{b}", bufs=2, space="PSUM")
            for e in range(nD):
                wp_col = wp_chunks[e // 4]
                j = e % 4
                p6 = ps6.tile([128, QL], F32, tag="p6")
                for h in range(nD):
                    nc.tensor.matmul(p6[:], wp_col[:, h, j * 128:(j + 1) * 128],
                                     out_uT[:, h, :],
                                     start=(h == 0), stop=(h == nD - 1))
                o2 = wpool.tile([128, QL], F32, tag="o2", bufs=3)
                nc.vector.tensor_mul(o2[:], p6[:], rzb[:])
                nc.scalar.add(o2[:], o2[:], bp_t[:, e:e + 1])
                nc.sync.dma_start(out_o[b, e], o2[:])
            ps6.release()
            wpool.release()
            oupool.release()
            persist.release()

        cpool.release()

    nc.compile()
    return nc


def _get_nc():
    global _compiled_nc
    if _compiled_nc is None:
        _compiled_nc = _build_nc()
    return _compiled_nc


def kernel(k, q, memory_len, Wk, bk, Wq, bq, Wp, bp, weight):
    k = np.asarray(k, dtype=np.float32)
    q = np.asarray(q, dtype=np.float32)
    memory_len = np.asarray(memory_len)
    Wk = np.asarray(Wk, dtype=np.float32)
    bk_v = np.asarray(bk, dtype=np.float32)
    Wq = np.asarray(Wq, dtype=np.float32)
    bq_v = np.asarray(bq, dtype=np.float32)
    Wp = np.asarray(Wp, dtype=np.float32)
    bp_v = np.asarray(bp, dtype=np.float32)
    weight = np.asarray(weight, dtype=np.float32)

    nc = _get_nc()

    wkT = np.ascontiguousarray(Wk.T).reshape(nD, 128, D)
    wqT = np.ascontiguousarray(Wq.T).reshape(nD, 128, D)
    wgt = np.ascontiguousarray(weight).reshape(nD, 128, D)
    wpT = np.ascontiguousarray(Wp.T).reshape(nD, 128, D)
    bk_r = np.ascontiguousarray(bk_v.reshape(nD, 128).T)
    bq_r = np.ascontiguousarray(bq_v.reshape(nD, 128).T)
    bp_r = np.ascontiguousarray(bp_v.reshape(nD, 128).T)
    ones = np.ones((128, 128), dtype=np.float32)
    ident = np.eye(128, dtype=np.float32)

    # per-(batch, k-tile, partition) mask bias: 0 where k-pos < memory_len else -50
    kpos = np.arange(KL).reshape(nK, 128)  # [kt, p]
    maskbias = np.where(kpos[None] < memory_len[:, None, None], 0.0,
                        MASK_NEG).astype(np.float32)
    maskbias = np.ascontiguousarray(maskbias.transpose(0, 2, 1))  # [B, 128, nK]

    kT_all = np.ascontiguousarray(k.transpose(0, 2, 1)).reshape(B, nD, 128, KL)
    qT_all = np.ascontiguousarray(q.transpose(0, 2, 1)).reshape(B, nD, 128, QL)

    in_maps = []
    for c in range(NCORES):
        bs = slice(c * BPC, (c + 1) * BPC)
        in_maps.append({
            "kT": kT_all[bs],
            "qT": qT_all[bs],
            "wkT": wkT, "wqT": wqT, "wgt": wgt, "wpT": wpT,
            "bk": bk_r, "bq": bq_r, "bp": bp_r,
            "maskb": np.ascontiguousarray(maskbias[bs]),
            "ones": ones, "identr": ident,
        })

    res = run_bass_kernel_spmd(nc, in_maps, core_ids=list(range(NCORES)))
    global last_results
    last_results = res

    attn = np.empty((B, QL, KL), dtype=np.float32)
    out = np.empty((B, QL, D), dtype=np.float32)
    for c in range(NCORES):
        r = res.results[c]
        # attn_o[b, kt, p, q] -> attn[b, q, kt*128+p]
        attn[c * BPC:(c + 1) * BPC] = (
            r["attn_o"].reshape(BPC, KL, QL).transpose(0, 2, 1))
        # out_o[b, et, p, q] -> out[b, q, et*128+p]
        out[c * BPC:(c + 1) * BPC] = (
            r["out_o"].reshape(BPC, D, QL).transpose(0, 2, 1))
    return out, attn


# revision 31
# speedup vs baseline: 1.1248x; 1.1248x over previous
"""Sparse-attention Bass kernel for Trainium2 (8 NeuronCores, data-parallel over batch).

Per batch b:
    kx    = k @ Wk.T + bk                      [KL, H]
    qx    = q @ Wq.T + bq                      [QL, H]
    qw    = qx @ weight                        [QL, H]
    score = tanh(qw @ kx.T)                    [QL, KL]
    p     = exp(score + maskbias)              (maskbias = 0 / -50 prefix mask)
    attn  = p / colsum_k(p)                    [QL, KL]   (output 1)
    out   = attn @ (kx @ Wp.T) + bp            [QL, E]    (output 2, reassociated)

Everything on-chip is laid out so matmul contraction dims sit on SBUF
partitions; score/attn live transposed as [KL, QL] and the host transposes the
attn/out results back. Matmuls use float32r (fast fp32 path). B=16 batches,
2 per core.
"""
import sys

sys.path.insert(0, "/opt/trn_rl_repo")

import numpy as np

import concourse.bass as bass
import concourse.mybir as mybir
import concourse.tile as tile
from concourse import bacc
from concourse.bass_utils import run_bass_kernel_spmd

F32 = mybir.dt.float32
F32R = mybir.dt.float32r
AF = mybir.ActivationFunctionType

B, KL, QL, D = 16, 2048, 512, 1024
NCORES = 8
BPC = B // NCORES          # batches per core
nD = D // 128              # 8
nK = KL // 128             # 16
nQ = QL // 128             # 4
nC = KL // 512             # 4
MASK_NEG = -50.0

_compiled_nc = None
last_results = None


def _build_nc():
    nc = bacc.Bacc("TRN2", target_bir_lowering=False)

    # ---- DRAM I/O (per core) ----
    kT_d = nc.dram_tensor("kT", [BPC, nD, 128, KL], F32R, kind="ExternalInput")
    qT_d = nc.dram_tensor("qT", [BPC, nD, 128, QL], F32R, kind="ExternalInput")
    wkT_d = nc.dram_tensor("wkT", [nD, 128, D], F32R, kind="ExternalInput")
    wqT_d = nc.dram_tensor("wqT", [nD, 128, D], F32R, kind="ExternalInput")
    wgt_d = nc.dram_tensor("wgt", [nD, 128, D], F32R, kind="ExternalInput")
    wpT_d = nc.dram_tensor("wpT", [nD, 128, D], F32R, kind="ExternalInput")
    bk_d = nc.dram_tensor("bk", [128, nD], F32, kind="ExternalInput")
    bq_d = nc.dram_tensor("bq", [128, nD], F32, kind="ExternalInput")
    bp_d = nc.dram_tensor("bp", [128, nD], F32, kind="ExternalInput")
    maskb_d = nc.dram_tensor("maskb", [BPC, 128, nK], F32, kind="ExternalInput")
    ones_d = nc.dram_tensor("ones", [128, 128], F32R, kind="ExternalInput")
    identr_d = nc.dram_tensor("identr", [128, 128], F32R, kind="ExternalInput")
    # outputs, transposed: attnT[b, kt, p, q] = attn[b, q, kt*128+p]
    attn_o = nc.dram_tensor("attn_o", [BPC, nK, 128, QL], F32, kind="ExternalOutput")
    out_o = nc.dram_tensor("out_o", [BPC, nD, 128, QL], F32, kind="ExternalOutput")

    with tile.TileContext(nc) as tc:
        cpool = tc.alloc_tile_pool(name="consts", bufs=1)
        ones_t = cpool.tile([128, 128], F32R)
        gate_t = cpool.tile([1, 1], F32R)
        ident_r = cpool.tile([128, 128], F32R)
        nc.gpsimd.dma_start(ident_r[:], identr_d[:])
        bk_t = cpool.tile([128, nD], F32)
        bq_t = cpool.tile([128, nD], F32)
        bp_t = cpool.tile([128, nD], F32)
        nc.gpsimd.dma_start(ones_t[:], ones_d[:])
        nc.gpsimd.dma_start(bk_t[:], bk_d[:])
        nc.gpsimd.dma_start(bq_t[:], bq_d[:])
        nc.gpsimd.dma_start(bp_t[:], bp_d[:])

        for b in range(BPC):
            # persist across the whole batch iteration
            persist = tc.alloc_tile_pool(name=f"persist{b}", bufs=1)
            kxT = persist.tile([128, nD, KL], F32R)   # kx^T[h, k]
            qwT = persist.tile([128, nD, QL], F32R)   # qw^T[g, q]

            # Q-side (MM2/MM3) and K-side (MM1) are interleaved on the PE so
            # each one's DMA stalls are covered by the other's matmuls.
            # kT/wk stream on the gpsimd DMA queue, q-side on sync.
            kpool = tc.alloc_tile_pool(name=f"k{b}", bufs=1)
            qpool = tc.alloc_tile_pool(name=f"q{b}", bufs=1)
            psQ = tc.alloc_tile_pool(name=f"psq{b}", bufs=3, space="PSUM")
            psB = tc.alloc_tile_pool(name=f"psb{b}", bufs=1, space="PSUM")

            qT_td = []
            for dd in range(nD):
                qtd = qpool.tile([128, QL], F32R, tag="qtd", bufs=nD)
                nc.gpsimd.dma_start(qtd[:], qT_d[b, dd])
                qT_td.append(qtd)
            qxT = qpool.tile([128, nD, QL], F32R)

            def mm2_group(gi):
                h0 = gi * 4
                wq_a = qpool.tile([128, nD, 256], F32R, tag="wcol", bufs=4)
                nc.sync.dma_start(
                    wq_a[:], wqT_d[:, :, h0 * 128:h0 * 128 + 256].rearrange("n p k -> p n k"))
                wq_b = qpool.tile([128, nD, 256], F32R, tag="wcol", bufs=4)
                nc.sync.dma_start(
                    wq_b[:], wqT_d[:, :, h0 * 128 + 256:h0 * 128 + 512].rearrange("n p k -> p n k"))
                wq_ch = [wq_a, wq_b]
                pq = []
                for j in range(4):
                    pq_t = psQ.tile([128, QL], F32, tag=f"pq{j}", bufs=1)
                    pq.append(pq_t)
                for d in range(nD):
                    for j in range(4):
                        nc.tensor.matmul(pq[j][:],
                                         wq_ch[j // 2][:, d, (j % 2) * 128:(j % 2) * 128 + 128],
                                         qT_td[d][:],
                                         start=(d == 0), stop=(d == nD - 1))
                for j in range(4):
                    h = h0 + j
                    nc.scalar.add(qxT[:, h, :], pq[j][:], bq_t[:, h:h + 1])

            def mm3_chunk(gs):
                g0 = gs[0]
                for g in gs:
                    j = g - g0
                    if j % 2 == 0:
                        wg_ch = qpool.tile([128, nD, 256], F32R, tag="wcol", bufs=4)
                        nc.sync.dma_start(
                            wg_ch[:], wgt_d[:, :, g * 128:g * 128 + 256].rearrange("n p k -> p n k"))
                    pg = psQ.tile([128, QL], F32, tag=f"pq{j % 2}", bufs=1)
                    for h in range(nD):
                        nc.tensor.matmul(pg[:], wg_ch[:, h, (j % 2) * 128:(j % 2) * 128 + 128],
                                         qxT[:, h, :],
                                         start=(h == 0), stop=(h == nD - 1))
                    nc.vector.tensor_copy(qwT[:, g, :], pg[:])

            wk_hg = []

            def load_wk():
                for hg in range(2):
                    wk_g = kpool.tile([128, nD, 512], F32R, tag=f"wkhg{hg}", bufs=1)
                    nc.gpsimd.dma_start(
                        wk_g[:], wkT_d[:, :, hg * 512:(hg + 1) * 512].rearrange("n p k -> p n k"))
                    wk_hg.append(wk_g)

            def mm1_chunk(c):
                cs = slice(c * 512, (c + 1) * 512)
                kT_c = []
                for dh in range(2):
                    kT_h = kpool.tile([128, nD // 2, 512], F32R, tag="ktc", bufs=3)
                    nc.gpsimd.dma_start(
                        kT_h[:], kT_d[b, dh * 4:(dh + 1) * 4, :, cs].rearrange("n p k -> p n k"))
                    kT_c.append(kT_h)
                for hg in range(2):
                    for j in range(4):
                        h = hg * 4 + j
                        pk = psB.tile([128, 512], F32, tag=f"pk{j}", bufs=1)
                        for dh in range(2):
                            for dd in range(4):
                                nc.tensor.matmul(
                                    pk[:],
                                    wk_hg[hg][:, dh * 4 + dd, j * 128:(j + 1) * 128],
                                    kT_c[dh][:, dd, :],
                                    start=(dh == 0 and dd == 0),
                                    stop=(dh == 1 and dd == 3))
                        nc.scalar.add(kxT[:, h, cs], pk[:], bk_t[:, h:h + 1])

            mm2_group(0)
            load_wk()
            mm2_group(1)
            mm3_chunk(list(range(0, 4)))
            mm1_chunk(0)
            mm3_chunk(list(range(4, 8)))
            mm1_chunk(1)
            mm1_chunk(2)
            mm1_chunk(3)

            psB.release()
            psQ.release()
            qpool.release()
            kpool.release()

            # ---------------- MM4: score^T[k, q] + masked exp + Z
            oupool = tc.alloc_tile_pool(name=f"ou{b}", bufs=1)
            out_uT = oupool.tile([128, nD, QL], F32R)
            rzb = oupool.tile([128, QL], F32)          # 1/Z broadcast to 128 parts
            spool = tc.alloc_tile_pool(name=f"s{b}", bufs=1)
            p_t = spool.tile([128, nK, QL], F32R)      # exp(tanh(score)+maskb)
            maskb_t = spool.tile([128, nK], F32)
            nc.sync.dma_start(maskb_t[:], maskb_d[b])

            psZ = tc.alloc_tile_pool(name=f"psz{b}", bufs=1, space="PSUM")
            zacc = psZ.tile([128, QL], F32, tag="z", bufs=1)
            psS = tc.alloc_tile_pool(name=f"pss{b}", bufs=6, space="PSUM")
            def z_mm(kt):
                nc.tensor.matmul(zacc[:], ones_t[:], p_t[:, kt, :],
                                 start=(kt == 0), stop=(kt == nK - 1))

            for kt in range(nK):
                sc = psS.tile([128, QL], F32, tag="sc")
                ks = slice(kt * 128, (kt + 1) * 128)
                for g in range(nD):
                    nc.tensor.matmul(sc[:], kxT[:, g, ks], qwT[:, g, :],
                                     start=(g == 0), stop=(g == nD - 1))
                th = spool.tile([128, QL], F32, tag="th", bufs=2)
                nc.scalar.activation(th[:], sc[:], AF.Tanh)
                nc.scalar.activation(p_t[:, kt, :], th[:], AF.Exp,
                                     bias=maskb_t[:, kt:kt + 1])
                # Z matmul lags two tiles behind so the ACT chain has slack
                if kt >= 2:
                    z_mm(kt - 2)
            z_mm(nK - 2)
            z_mm(nK - 1)
            psS.release()
            nc.vector.reciprocal(rzb[:], zacc[:])

            # attn^T output = p * (1/Z)
            for kt in range(nK):
                stage = spool.tile([128, QL], F32, tag="stage", bufs=3)
                nc.vector.tensor_mul(stage[:], p_t[:, kt, :].bitcast(F32), rzb[:])
                nc.sync.dma_start(attn_o[b, kt], stage[:])

            wp_chunks = []
            for ch in range(2):
                wp_c = oupool.tile([128, nD, 512], F32R, tag="wpc", bufs=2)
                nc.sync.dma_start(
                    wp_c[:], wpT_d[:, :, ch * 512:(ch + 1) * 512].rearrange("n p k -> p n k"))
                wp_chunks.append(wp_c)

            # ---------------- out_u^T[h,q] = sum_k kx[k,h] p[k,q]  (kx via PE
            # transpose of kx^T, two h-halves), then out2^T = Wp^T-contract + bp
            kxpool = tc.alloc_tile_pool(name=f"kx{b}", bufs=1)
            psT5 = tc.alloc_tile_pool(name=f"pst{b}", bufs=1, space="PSUM")
            for hh in range(2):
                kx = kxpool.tile([128, nK, 512], F32R, tag="kx", bufs=1)
                for kt in range(nK):
                    ks = slice(kt * 128, (kt + 1) * 128)
                    tp = psT5.tile([128, 512], F32R, tag="tp", bufs=3)
                    for j in range(4):
                        nc.tensor.transpose(tp[:, j * 128:(j + 1) * 128],
                                            kxT[:, hh * 4 + j, ks], ident_r[:])
                    if kt % 2 == 0:
                        nc.vector.tensor_copy(kx[:, kt, :], tp[:])
                    else:
                        nc.scalar.copy(kx[:, kt, :], tp[:])
                po = []
                for j in range(4):
                    po_t = psT5.tile([128, QL], F32, tag=f"po{j}", bufs=1)
                    po.append(po_t)
                for kt in range(nK):
                    for j in range(4):
                        nc.tensor.matmul(po[j][:], kx[:, kt, j * 128:(j + 1) * 128],
                                         p_t[:, kt, :],
                                         start=(kt == 0), stop=(kt == nK - 1))
                for j in range(4):
                    h = hh * 4 + j
                    if j % 2 == 0:
                        nc.vector.tensor_copy(out_uT[:, h, :], po[j][:])
                    else:
                        nc.scalar.copy(out_uT[:, h, :], po[j][:])
            psT5.release()
            kxpool.release()
            psZ.release()
            spool.release()

            # out2^T[e, q] = sum_h WpT[h, e] out_u^T[h, q], * rz + bp
            wpool = tc.alloc_tile_pool(name=f"wp{b}", bufs=1)
            ps6 = tc.alloc_tile_pool(name=f"ps6{b}", bufs=2, space="PSUM")
            for e in range(nD):
                wp_col = wp_chunks[e // 4]
                j = e % 4
                p6 = ps6.tile([128, QL], F32, tag="p6")
                for h in range(nD):
                    nc.tensor.matmul(p6[:], wp_col[:, h, j * 128:(j + 1) * 128],
                                     out_uT[:, h, :],
                                     start=(h == 0), stop=(h == nD - 1))
                o2 = wpool.tile([128, QL], F32, tag="o2", bufs=3)
                nc.vector.tensor_mul(o2[:], p6[:], rzb[:])
                nc.scalar.add(o2[:], o2[:], bp_t[:, e:e + 1])
                nc.sync.dma_start(out_o[b, e], o2[:])
            ps6.release()
            wpool.release()
            oupool.release()
            persist.release()

        cpool.release()

    nc.compile()
    return nc


def _get_nc():
    global _compiled_nc
    if _compiled_nc is None:
        _compiled_nc = _build_nc()
    return _compiled_nc


def kernel(k, q, memory_len, Wk, bk, Wq, bq, Wp, bp, weight):
    k = np.asarray(k, dtype=np.float32)
    q = np.asarray(q, dtype=np.float32)
    memory_len = np.asarray(memory_len)
    Wk = np.asarray(Wk, dtype=np.float32)
    bk_v = np.asarray(bk, dtype=np.float32)
    Wq = np.asarray(Wq, dtype=np.float32)
    bq_v = np.asarray(bq, dtype=np.float32)
    Wp = np.asarray(Wp, dtype=np.float32)
    bp_v = np.asarray(bp, dtype=np.float32)
    weight = np.asarray(weight, dtype=np.float32)

    nc = _get_nc()

    wkT = np.ascontiguousarray(Wk.T).reshape(nD, 128, D)
    wqT = np.ascontiguousarray(Wq.T).reshape(nD, 128, D)
    wgt = np.ascontiguousarray(weight).reshape(nD, 128, D)
    wpT = np.ascontiguousarray(Wp.T).reshape(nD, 128, D)
    bk_r = np.ascontiguousarray(bk_v.reshape(nD, 128).T)
    bq_r = np.ascontiguousarray(bq_v.reshape(nD, 128).T)
    bp_r = np.ascontiguousarray(bp_v.reshape(nD, 128).T)
    ones = np.ones((128, 128), dtype=np.float32)
    ident = np.eye(128, dtype=np.float32)

    # per-(batch, k-tile, partition) mask bias: 0 where k-pos < memory_len else -50
    kpos = np.arange(KL).reshape(nK, 128)  # [kt, p]
    maskbias = np.where(kpos[None] < memory_len[:, None, None], 0.0,
                        MASK_NEG).astype(np.float32)
    maskbias = np.ascontiguousarray(maskbias.transpose(0, 2, 1))  # [B, 128, nK]

    kT_all = np.ascontiguousarray(k.transpose(0, 2, 1)).reshape(B, nD, 128, KL)
    qT_all = np.ascontiguousarray(q.transpose(0, 2, 1)).reshape(B, nD, 128, QL)

    in_maps = []
    for c in range(NCORES):
        bs = slice(c * BPC, (c + 1) * BPC)
        in_maps.append({
            "kT": kT_all[bs],
            "qT": qT_all[bs],
            "wkT": wkT, "wqT": wqT, "wgt": wgt, "wpT": wpT,
            "bk": bk_r, "bq": bq_r, "bp": bp_r,
            "maskb": np.ascontiguousarray(maskbias[bs]),
            "ones": ones, "identr": ident,
        })

    res = run_bass_kernel_spmd(nc, in_maps, core_ids=list(range(NCORES)))
    global last_results
    last_results = res

    attn = np.empty((B, QL, KL), dtype=np.float32)
    out = np.empty((B, QL, D), dtype=np.float32)
    for c in range(NCORES):
        r = res.results[c]
        # attn_o[b, kt, p, q] -> attn[b, q, kt*128+p]
        attn[c * BPC:(c + 1) * BPC] = (
            r["attn_o"].reshape(BPC, KL, QL).transpose(0, 2, 1))
        # out_o[b, et, p, q] -> out[b, q, et*128+p]
        out[c * BPC:(c + 1) * BPC] = (
            r["out_o"].reshape(BPC, D, QL).transpose(0, 2, 1))
    return out, attn


# revision 32
# speedup vs baseline: 1.1699x; 1.0400x over previous
"""Sparse-attention Bass kernel for Trainium2 (8 NeuronCores, data-parallel over batch).

Per batch b:
    kx    = k @ Wk.T + bk                      [KL, H]
    qx    = q @ Wq.T + bq                      [QL, H]
    qw    = qx @ weight                        [QL, H]
    score = tanh(qw @ kx.T)                    [QL, KL]
    p     = exp(score + maskbias)              (maskbias = 0 / -50 prefix mask)
    attn  = p / colsum_k(p)                    [QL, KL]   (output 1)
    out   = attn @ (kx @ Wp.T) + bp            [QL, E]    (output 2, reassociated)

Everything on-chip is laid out so matmul contraction dims sit on SBUF
partitions; score/attn live transposed as [KL, QL] and the host transposes the
attn/out results back. Matmuls use float32r (fast fp32 path). B=16 batches,
2 per core.
"""
import sys

sys.path.insert(0, "/opt/trn_rl_repo")

import numpy as np

import concourse.bass as bass
import concourse.mybir as mybir
import concourse.tile as tile
from concourse import bacc
from concourse.bass_utils import run_bass_kernel_spmd

F32 = mybir.dt.float32
F32R = mybir.dt.float32r
AF = mybir.ActivationFunctionType

B, KL, QL, D = 16, 2048, 512, 1024
NCORES = 8
BPC = B // NCORES          # batches per core
nD = D // 128              # 8
nK = KL // 128             # 16
nQ = QL // 128             # 4
nC = KL // 512             # 4
MASK_NEG = -50.0

_compiled_nc = None
last_results = None


def _build_nc():
    nc = bacc.Bacc("TRN2", target_bir_lowering=False)

    # ---- DRAM I/O (per core) ----
    kT_d = nc.dram_tensor("kT", [BPC, nD, 128, KL], F32R, kind="ExternalInput")
    qT_d = nc.dram_tensor("qT", [BPC, nD, 128, QL], F32R, kind="ExternalInput")
    wkT_d = nc.dram_tensor("wkT", [nD, 128, D], F32R, kind="ExternalInput")
    wqT_d = nc.dram_tensor("wqT", [nD, 128, D], F32R, kind="ExternalInput")
    wgt_d = nc.dram_tensor("wgt", [nD, 128, D], F32R, kind="ExternalInput")
    wpT_d = nc.dram_tensor("wpT", [nD, 128, D], F32R, kind="ExternalInput")
    bk_d = nc.dram_tensor("bk", [128, nD], F32, kind="ExternalInput")
    bq_d = nc.dram_tensor("bq", [128, nD], F32, kind="ExternalInput")
    bp_d = nc.dram_tensor("bp", [128, nD], F32, kind="ExternalInput")
    maskb_d = nc.dram_tensor("maskb", [BPC, 128, nK], F32, kind="ExternalInput")
    ones_d = nc.dram_tensor("ones", [128, 128], F32R, kind="ExternalInput")
    identr_d = nc.dram_tensor("identr", [128, 128], F32R, kind="ExternalInput")
    # outputs, transposed: attnT[b, kt, p, q] = attn[b, q, kt*128+p]
    attn_o = nc.dram_tensor("attn_o", [BPC, nK, 128, QL], F32, kind="ExternalOutput")
    out_o = nc.dram_tensor("out_o", [BPC, nD, 128, QL], F32, kind="ExternalOutput")

    with tile.TileContext(nc) as tc:
        cpool = tc.alloc_tile_pool(name="consts", bufs=1)
        ones_t = cpool.tile([128, 128], F32R)
        gate_t = cpool.tile([1, 1], F32R)
        ident_r = cpool.tile([128, 128], F32R)
        nc.gpsimd.dma_start(ident_r[:], identr_d[:])
        bk_t = cpool.tile([128, nD], F32)
        bq_t = cpool.tile([128, nD], F32)
        bp_t = cpool.tile([128, nD], F32)
        nc.gpsimd.dma_start(ones_t[:], ones_d[:])
        nc.gpsimd.dma_start(bk_t[:], bk_d[:])
        nc.gpsimd.dma_start(bq_t[:], bq_d[:])
        nc.gpsimd.dma_start(bp_t[:], bp_d[:])

        for b in range(BPC):
            # persist across the whole batch iteration
            persist = tc.alloc_tile_pool(name=f"persist{b}", bufs=1)
            kxT = persist.tile([128, nD, KL], F32R)   # kx^T[h, k]
            qwT = persist.tile([128, nD, QL], F32R)   # qw^T[g, q]

            # Q-side (MM2/MM3) and K-side (MM1) are interleaved on the PE so
            # each one's DMA stalls are covered by the other's matmuls.
            # kT/wk stream on the gpsimd DMA queue, q-side on sync.
            kpool = tc.alloc_tile_pool(name=f"k{b}", bufs=1)
            qpool = tc.alloc_tile_pool(name=f"q{b}", bufs=1)
            psQ = tc.alloc_tile_pool(name=f"psq{b}", bufs=3, space="PSUM")
            psB = tc.alloc_tile_pool(name=f"psb{b}", bufs=1, space="PSUM")

            qT_td = []
            for dd in range(nD):
                qtd = qpool.tile([128, QL], F32R, tag="qtd", bufs=nD)
                nc.gpsimd.dma_start(qtd[:], qT_d[b, dd])
                qT_td.append(qtd)
            qxT = qpool.tile([128, nD, QL], F32R)

            def mm2_group(gi):
                h0 = gi * 4
                wq_c = []
                for j in range(4):
                    h = h0 + j
                    wq_t = qpool.tile([128, nD, 128], F32R, tag="wcol", bufs=8)
                    nc.sync.dma_start(
                        wq_t[:], wqT_d[:, :, h * 128:(h + 1) * 128].rearrange("n p k -> p n k"))
                    wq_c.append(wq_t)
                pq = []
                for j in range(4):
                    pq_t = psQ.tile([128, QL], F32, tag=f"pq{j}", bufs=1)
                    pq.append(pq_t)
                for d in range(nD):
                    for j in range(4):
                        nc.tensor.matmul(pq[j][:], wq_c[j][:, d, :], qT_td[d][:],
                                         start=(d == 0), stop=(d == nD - 1))
                for j in range(4):
                    h = h0 + j
                    nc.scalar.add(qxT[:, h, :], pq[j][:], bq_t[:, h:h + 1])

            def mm3_chunk(gs):
                g0 = gs[0]
                for g in gs:
                    j = g - g0
                    wg_c = qpool.tile([128, nD, 128], F32R, tag="wcol", bufs=8)
                    nc.sync.dma_start(
                        wg_c[:], wgt_d[:, :, g * 128:(g + 1) * 128].rearrange("n p k -> p n k"))
                    pg = psQ.tile([128, QL], F32, tag=f"pq{j % 2}", bufs=1)
                    for h in range(nD):
                        nc.tensor.matmul(pg[:], wg_c[:, h, :], qxT[:, h, :],
                                         start=(h == 0), stop=(h == nD - 1))
                    nc.vector.tensor_copy(qwT[:, g, :], pg[:])

            wk_hg = []

            def load_wk():
                for hg in range(2):
                    wk_g = kpool.tile([128, nD, 512], F32R, tag=f"wkhg{hg}", bufs=1)
                    nc.gpsimd.dma_start(
                        wk_g[:], wkT_d[:, :, hg * 512:(hg + 1) * 512].rearrange("n p k -> p n k"))
                    wk_hg.append(wk_g)

            def mm1_chunk(c):
                cs = slice(c * 512, (c + 1) * 512)
                kT_c = []
                for dh in range(2):
                    kT_h = kpool.tile([128, nD // 2, 512], F32R, tag="ktc", bufs=3)
                    nc.gpsimd.dma_start(
                        kT_h[:], kT_d[b, dh * 4:(dh + 1) * 4, :, cs].rearrange("n p k -> p n k"))
                    kT_c.append(kT_h)
                for hg in range(2):
                    for j in range(4):
                        h = hg * 4 + j
                        pk = psB.tile([128, 512], F32, tag=f"pk{j}", bufs=1)
                        for dh in range(2):
                            for dd in range(4):
                                nc.tensor.matmul(
                                    pk[:],
                                    wk_hg[hg][:, dh * 4 + dd, j * 128:(j + 1) * 128],
                                    kT_c[dh][:, dd, :],
                                    start=(dh == 0 and dd == 0),
                                    stop=(dh == 1 and dd == 3))
                        nc.scalar.add(kxT[:, h, cs], pk[:], bk_t[:, h:h + 1])

            mm2_group(0)
            load_wk()
            mm2_group(1)
            mm3_chunk(list(range(0, 4)))
            mm1_chunk(0)
            mm3_chunk(list(range(4, 8)))
            mm1_chunk(1)
            mm1_chunk(2)
            mm1_chunk(3)

            psB.release()
            psQ.release()
            qpool.release()
            kpool.release()

            # ---------------- MM4: score^T[k, q] + masked exp + Z
            oupool = tc.alloc_tile_pool(name=f"ou{b}", bufs=1)
            out_uT = oupool.tile([128, nD, QL], F32R)
            rzb = oupool.tile([128, QL], F32)          # 1/Z broadcast to 128 parts
            spool = tc.alloc_tile_pool(name=f"s{b}", bufs=1)
            p_t = spool.tile([128, nK, QL], F32R)      # exp(tanh(score)+maskb)
            maskb_t = spool.tile([128, nK], F32)
            nc.sync.dma_start(maskb_t[:], maskb_d[b])

            psZ = tc.alloc_tile_pool(name=f"psz{b}", bufs=1, space="PSUM")
            zacc = psZ.tile([128, QL], F32, tag="z", bufs=1)
            psS = tc.alloc_tile_pool(name=f"pss{b}", bufs=6, space="PSUM")
            def z_mm(kt):
                nc.tensor.matmul(zacc[:], ones_t[:], p_t[:, kt, :],
                                 start=(kt == 0), stop=(kt == nK - 1))

            for kt in range(nK):
                sc = psS.tile([128, QL], F32, tag="sc")
                ks = slice(kt * 128, (kt + 1) * 128)
                for g in range(nD):
                    nc.tensor.matmul(sc[:], kxT[:, g, ks], qwT[:, g, :],
                                     start=(g == 0), stop=(g == nD - 1))
                th = spool.tile([128, QL], F32, tag="th", bufs=2)
                nc.scalar.activation(th[:], sc[:], AF.Tanh)
                nc.scalar.activation(p_t[:, kt, :], th[:], AF.Exp,
                                     bias=maskb_t[:, kt:kt + 1])
                # Z matmul lags two tiles behind so the ACT chain has slack
                if kt >= 2:
                    z_mm(kt - 2)
            z_mm(nK - 2)
            z_mm(nK - 1)
            psS.release()
            nc.vector.reciprocal(rzb[:], zacc[:])

            # attn^T output = p * (1/Z)
            for kt in range(nK):
                stage = spool.tile([128, QL], F32, tag="stage", bufs=3)
                nc.vector.tensor_mul(stage[:], p_t[:, kt, :].bitcast(F32), rzb[:])
                nc.sync.dma_start(attn_o[b, kt], stage[:])

            wp_chunks = []
            for ch in range(2):
                wp_c = oupool.tile([128, nD, 512], F32R, tag="wpc", bufs=2)
                nc.sync.dma_start(
                    wp_c[:], wpT_d[:, :, ch * 512:(ch + 1) * 512].rearrange("n p k -> p n k"))
                wp_chunks.append(wp_c)

            # ---------------- out_u^T[h,q] = sum_k kx[k,h] p[k,q]  (kx via PE
            # transpose of kx^T, two h-halves), then out2^T = Wp^T-contract + bp
            kxpool = tc.alloc_tile_pool(name=f"kx{b}", bufs=1)
            psT5 = tc.alloc_tile_pool(name=f"pst{b}", bufs=1, space="PSUM")
            for hh in range(2):
                kx = kxpool.tile([128, nK, 512], F32R, tag="kx", bufs=1)
                for kt in range(nK):
                    ks = slice(kt * 128, (kt + 1) * 128)
                    tp = psT5.tile([128, 512], F32R, tag="tp", bufs=3)
                    for j in range(4):
                        nc.tensor.transpose(tp[:, j * 128:(j + 1) * 128],
                                            kxT[:, hh * 4 + j, ks], ident_r[:])
                    if kt % 2 == 0:
                        nc.vector.tensor_copy(kx[:, kt, :], tp[:])
                    else:
                        nc.scalar.copy(kx[:, kt, :], tp[:])
                po = []
                for j in range(4):
                    po_t = psT5.tile([128, QL], F32, tag=f"po{j}", bufs=1)
                    po.append(po_t)
                for kt in range(nK):
                    for j in range(4):
                        nc.tensor.matmul(po[j][:], kx[:, kt, j * 128:(j + 1) * 128],
                                         p_t[:, kt, :],
                                         start=(kt == 0), stop=(kt == nK - 1))
                for j in range(4):
                    h = hh * 4 + j
                    if j % 2 == 0:
                        nc.vector.tensor_copy(out_uT[:, h, :], po[j][:])
                    else:
                        nc.scalar.copy(out_uT[:, h, :], po[j][:])
            psT5.release()
            kxpool.release()
            psZ.release()
            spool.release()

            # out2^T[e, q] = sum_h WpT[h, e] out_u^T[h, q], * rz + bp
            wpool = tc.alloc_tile_pool(name=f"wp{b}", bufs=1)
            ps6 = tc.alloc_tile_pool(name=f"ps6{b}", bufs=2, space="PSUM")
            for e in range(nD):
                wp_col = wp_chunks[e // 4]
                j = e % 4
                p6 = ps6.tile([128, QL], F32, tag="p6")
                for h in range(nD):
                    nc.tensor.matmul(p6[:], wp_col[:, h, j * 128:(j + 1) * 128],
                                     out_uT[:, h, :],
                                     start=(h == 0), stop=(h == nD - 1))
                o2 = wpool.tile([128, QL], F32, tag="o2", bufs=3)
                nc.vector.tensor_mul(o2[:], p6[:], rzb[:])
                nc.scalar.add(o2[:], o2[:], bp_t[:, e:e + 1])
                nc.sync.dma_start(out_o[b, e], o2[:])
            ps6.release()
            wpool.release()
            oupool.release()
            persist.release()

        cpool.release()

    nc.compile()
    return nc


def _get_nc():
    global _compiled_nc
    if _compiled_nc is None:
        _compiled_nc = _build_nc()
    return _compiled_nc


def kernel(k, q, memory_len, Wk, bk, Wq, bq, Wp, bp, weight):
    k = np.asarray(k, dtype=np.float32)
    q = np.asarray(q, dtype=np.float32)
    memory_len = np.asarray(memory_len)
    Wk = np.asarray(Wk, dtype=np.float32)
    bk_v = np.asarray(bk, dtype=np.float32)
    Wq = np.asarray(Wq, dtype=np.float32)
    bq_v = np.asarray(bq, dtype=np.float32)
    Wp = np.asarray(Wp, dtype=np.float32)
    bp_v = np.asarray(bp, dtype=np.float32)
    weight = np.asarray(weight, dtype=np.float32)

    nc = _get_nc()

    wkT = np.ascontiguousarray(Wk.T).reshape(nD, 128, D)
    wqT = np.ascontiguousarray(Wq.T).reshape(nD, 128, D)
    wgt = np.ascontiguousarray(weight).reshape(nD, 128, D)
    wpT = np.ascontiguousarray(Wp.T).reshape(nD, 128, D)
    bk_r = np.ascontiguousarray(bk_v.reshape(nD, 128).T)
    bq_r = np.ascontiguousarray(bq_v.reshape(nD, 128).T)
    bp_r = np.ascontiguousarray(bp_v.reshape(nD, 128).T)
    ones = np.ones((128, 128), dtype=np.float32)
    ident = np.eye(128, dtype=np.float32)

    # per-(batch, k-tile, partition) mask bias: 0 where k-pos < memory_len else -50
    kpos = np.arange(KL).reshape(nK, 128)  # [kt, p]
    maskbias = np.where(kpos[None] < memory_len[:, None, None], 0.0,
                        MASK_NEG).astype(np.float32)
    maskbias = np.ascontiguousarray(maskbias.transpose(0, 2, 1))  # [B, 128, nK]

    kT_all = np.ascontiguousarray(k.transpose(0, 2, 1)).reshape(B, nD, 128, KL)
    qT_all = np.ascontiguousarray(q.transpose(0, 2, 1)).reshape(B, nD, 128, QL)

    in_maps = []
    for c in range(NCORES):
        bs = slice(c * BPC, (c + 1) * BPC)
        in_maps.append({
            "kT": kT_all[bs],
            "qT": qT_all[bs],
            "wkT": wkT, "wqT": wqT, "wgt": wgt, "wpT": wpT,
            "bk": bk_r, "bq": bq_r, "bp": bp_r,
            "maskb": np.ascontiguousarray(maskbias[bs]),
            "ones": ones, "identr": ident,
        })

    res = run_bass_kernel_spmd(nc, in_maps, core_ids=list(range(NCORES)))
    global last_results
    last_results = res

    attn = np.empty((B, QL, KL), dtype=np.float32)
    out = np.empty((B, QL, D), dtype=np.float32)
    for c in range(NCORES):
        r = res.results[c]
        # attn_o[b, kt, p, q] -> attn[b, q, kt*128+p]
        attn[c * BPC:(c + 1) * BPC] = (
            r["attn_o"].reshape(BPC, KL, QL).transpose(0, 2, 1))
        # out_o[b, et, p, q] -> out[b, q, et*128+p]
        out[c * BPC:(c + 1) * BPC] = (
            r["out_o"].reshape(BPC, D, QL).transpose(0, 2, 1))
    return out, attn
